# Initial kernel scaffold
#
"""Trainium2 Bass kernel for nn_ADAM_17575006175639 (deformable-conv attention block).

Strategy (8 NeuronCores, data-parallel over batch, 2 samples/core):
  - xr = w_red @ x on PE (bf16)
  - offset conv 7x7 on PE (49 shifted K=28 matmuls over zero-padded xr)
  - deformable bilinear sampling: per-(tap,pixel) row gather from a DRAM table
    [v | Dy | Dx | Dxy] (difference basis) via gpsimd dma_gather (4 SWDGE
    queues), landing pixel-major so tent weights apply as per-pixel broadcasts
    on DVE; tap-contraction (w_dcn) on PE via per-group PE transposes
  - BN stats all-reduced across the 8 cores (tiny collective)
  - SE gate + mix + w_res restore on PE/ACT/DVE
"""
import os
import numpy as np
import ml_dtypes

import concourse.bass as bass
import concourse.bacc as bacc
import concourse.tile as tile
from concourse import mybir
from concourse.bass_utils import run_bass_kernel_spmd

F32 = mybir.dt.float32
BF16 = mybir.dt.bfloat16
I16 = mybir.dt.int16
AF = mybir.ActivationFunctionType
OP = mybir.AluOpType

B, C, H, W = 16, 512, 64, 64
CR, K, PAD = 28, 7, 3
NCORES = 8
SLOC = B // NCORES            # samples per core
P = H * W                     # 4096 pixels
PADW = 10                     # zero-pad margin (|offset| must stay < 7)
PW = H + 2 * PADW             # 84 padded width
PP = PW * PW                  # 7056 padded pixels
PPR = 7168                    # padded to 56*128 for transpose chunks
NB = P // 128                 # 32 pixel blocks
KT = K * K                    # 49 taps
NG = 25                       # tap groups of 2 (50 >= 49)
GSZ = 2                       # taps per group
CPAD = 32                     # padded channel dim in Phi
NCH = 4                       # gather chunks per tap (1024 idx each)
EL = 128                      # table row elements (bf16) = 256B
WSE, WSA = None, None         # folded on host

_cache = {}


def _to_bf(a):
    return np.ascontiguousarray(a.astype(ml_dtypes.bfloat16))


def host_consts(w_red, w_se1, b_se1, w_se2, b_se2, w_off, b_off, w_dcn,
                bn_gamma, bn_beta, w_res, weight_se, weight_sa):
    c = {}
    c["wredT"] = _to_bf(w_red.T.reshape(4, 128, CR))                  # [4,128,28]
    c["woffT"] = _to_bf(np.transpose(w_off, (2, 3, 1, 0)).reshape(KT, CR, 2 * KT)
                        .transpose(1, 0, 2))  # [28,49,98]
    # wd rows: group g, kk in 0..GSZ-1 -> tap k=GSZ*g+kk, channel c' (pad zero)
    wd = np.zeros((NG, GSZ, CPAD, CR), np.float32)
    for g in range(NG):
        for kk in range(GSZ):
            k = GSZ * g + kk
            if k < KT:
                kh, kw = k // K, k % K
                wd[g, kk, :CR, :] = w_dcn[:, :, kh, kw].T             # [c_in, o]
    c["wdT"] = _to_bf(wd.reshape(NG, GSZ * CPAD, CR))                 # [25,64,28]
    c["wresT"] = _to_bf(w_res.T.reshape(CR, 4, 128))                  # [28,4,128]
    c["wse1T"] = _to_bf(w_se1.T)                                      # [28,28]
    c["wse2T"] = _to_bf(w_se2.T)
    c["bse1"] = np.ascontiguousarray(b_se1.reshape(CR, 1).astype(np.float32))
    c["bse2"] = np.ascontiguousarray(b_se2.reshape(CR, 1).astype(np.float32))
    c["gamma_r"] = np.ascontiguousarray(bn_gamma.reshape(1, CR).astype(np.float32))
    c["beta_r"] = np.ascontiguousarray(bn_beta.reshape(1, CR).astype(np.float32))
    c["idn_b"] = _to_bf(np.eye(128, dtype=np.float32))
    c["idn_f"] = np.eye(128, dtype=np.float32)
    c["ones128"] = np.ones((128, 1), np.float32)
    c["ones_r"] = np.ones((1, 128), np.float32)
    # grids: pixel pi = 128*b + p ; i = pi//64, j = pi%64
    p_idx = np.arange(128)
    b_idx = np.arange(NB)
    pi = (128 * b_idx[None, :, None] + p_idx[:, None, None])          # [128,32,1]
    ii = pi // W
    jj = pi % W
    kh = (np.arange(KT) // K)[None, None, :]
    kw = (np.arange(KT) % K)[None, None, :]
    gy = ii + kh + (PADW - PAD) + b_off[0::2][None, None, :] - 0.5
    gx = jj + kw + (PADW - PAD) + b_off[1::2][None, None, :] - 0.5
    c["grid_y"] = _to_bf(gy)          # [128,32,49] (exactly representable)
    c["grid_x"] = _to_bf(gx)
    assert np.all(c["grid_y"].astype(np.float64) == gy)
    assert np.all(c["grid_x"].astype(np.float64) == gx)
    wse = float(weight_se.reshape(-1)[0])
    wsa = float(weight_sa.reshape(-1)[0])
    den = wse + wsa + 1e-6
    c["wse_f"] = wse / den
    c["wsa_f"] = wsa / den
    return c


def build(local_bn=False, wse_f=0.4999995, wsa_f=0.4999995):
    """Builds the SPMD program. local_bn: replica groups of 1 (for sim)."""
    nc = bacc.Bacc("TRN2", target_bir_lowering=False, debug=False,
                   num_devices=NCORES, num_swdge_queues=4)
    x_in = nc.dram_tensor("x", [SLOC, C, P], F32, kind="ExternalInput")
    cN = {}
    for nm, shape, dt in [
        ("wredT", [4, 128, CR], BF16), ("woffT", [CR, KT, 2 * KT], BF16),
        ("wdT", [NG, GSZ * CPAD, CR], BF16), ("wresT", [CR, 4, 128], BF16),
        ("wse1T", [CR, CR], BF16), ("wse2T", [CR, CR], BF16),
        ("bse1", [CR, 1], F32), ("bse2", [CR, 1], F32),
        ("gamma_r", [1, CR], F32), ("beta_r", [1, CR], F32),
        ("idn_b", [128, 128], BF16), ("idn_f", [128, 128], F32),
        ("ones128", [128, 1], F32), ("ones_r", [1, 128], F32),
        ("grid_y", [128, NB, KT], BF16), ("grid_x", [128, NB, KT], BF16),
    ]:
        cN[nm] = nc.dram_tensor(nm, shape, dt, kind="ExternalInput")
    out_d = nc.dram_tensor("out", [SLOC, C, P], F32, kind="ExternalOutput")
    dbg_xr = nc.dram_tensor("dbg_xr", [SLOC, 128, NB * CR], F32, kind="ExternalOutput")
    dbg_off = nc.dram_tensor("dbg_off", [SLOC, 128, NB * 2 * KT], F32, kind="ExternalOutput")
    dbg_sa = nc.dram_tensor("dbg_sa", [SLOC, 128, NB * CR], F32, kind="ExternalOutput")
    dbg_uy = nc.dram_tensor("dbg_uy", [SLOC, 128, NB * KT], F32, kind="ExternalOutput")
    dbg_q0 = nc.dram_tensor("dbg_q0", [SLOC, 128, NB * KT], I16, kind="ExternalOutput")
    dbg_phi = nc.dram_tensor("dbg_phi", [SLOC, 128, NB * GSZ * CPAD], BF16, kind="ExternalOutput")
    dbg_ux = nc.dram_tensor("dbg_ux", [SLOC, 128, NB * KT], BF16, kind="ExternalOutput")
    table = nc.dram_tensor("tbl", [SLOC, PPR, EL], BF16, kind="ExternalOutput")
    q0_dram = nc.dram_tensor("q0scr", [SLOC, 128 * NB * KT], I16, kind="Internal")
    ccin = nc.dram_tensor("ccin", [1, 64], F32, kind="Internal")
    ccout = nc.dram_tensor("ccout", [1, 64], F32, kind="Internal",
                           addr_space="Shared")
    groups = [[i] for i in range(NCORES)] if local_bn else [list(range(NCORES))]

    with tile.TileContext(nc) as tc:
        import contextlib
        ctx = contextlib.ExitStack()
        with ctx:
            sing = ctx.enter_context(tc.tile_pool(name="sing", bufs=1))
            persist = ctx.enter_context(tc.tile_pool(name="persist", bufs=1))
            persist2 = ctx.enter_context(tc.tile_pool(name="persist2", bufs=2))
            small = ctx.enter_context(tc.tile_pool(name="small", bufs=1))
            gpool = ctx.enter_context(tc.tile_pool(name="g", bufs=2))
            ppa = ctx.enter_context(tc.tile_pool(name="ppa", bufs=2, space="PSUM"))
            pps = ctx.enter_context(tc.tile_pool(name="pps", bufs=2, space="PSUM"))

            # ---- constants to SBUF ----
            wredT = sing.tile([128, 4, CR], BF16)
            nc.sync.dma_start(out=wredT[:],
                              in_=cN["wredT"].ap().rearrange("c p m -> p c m"))
            wdT = sing.tile([GSZ * CPAD, NG, CR], BF16)
            nc.sync.dma_start(out=wdT[:],
                              in_=cN["wdT"].ap().rearrange("g p m -> p g m"))

            def _load(nm, shape, dt):
                t = sing.tile(shape, dt, name=nm + "_t")
                nc.sync.dma_start(out=t[:], in_=cN[nm].ap())
                return t
            woff_t = _load("woffT", [CR, KT, 2 * KT], BF16)
            wresT = _load("wresT", [CR, 4, 128], BF16)
            wse1T = _load("wse1T", [CR, CR], BF16)
            wse2T = _load("wse2T", [CR, CR], BF16)
            bse1 = _load("bse1", [CR, 1], F32)
            bse2 = _load("bse2", [CR, 1], F32)
            gamma_r = _load("gamma_r", [1, CR], F32)
            beta_r = _load("beta_r", [1, CR], F32)
            idn_b = _load("idn_b", [128, 128], BF16)
            idn_f = _load("idn_f", [128, 128], F32)
            ones128 = _load("ones128", [128, 1], F32)
            ones_r = _load("ones_r", [1, 128], F32)
            grid_y = _load("grid_y", [128, NB, KT], BF16)
            grid_x = _load("grid_x", [128, NB, KT], BF16)

            saT_all, xrp_all, se_all, stat_s = [], [], [], []

            for s in range(SLOC):
                ctx_s = contextlib.ExitStack()
                with ctx_s:
                    mid = ctx_s.enter_context(
                        tc.tile_pool(name=f"mid{s}", bufs=1))
                    ctx_p = contextlib.ExitStack()
                    ctx_p.__enter__()
                    pxp = ctx_p.enter_context(
                        tc.tile_pool(name=f"pxp{s}", bufs=1))
                    ctx_e = contextlib.ExitStack()
                    ctx_e.__enter__()
                    early = ctx_e.enter_context(
                        tc.tile_pool(name=f"early{s}", bufs=1))

                    # ---- xr matmul (stream x in halves, cast-DMA f32->bf16) ----
                    xpad = pxp.tile([CR, PPR + 4], BF16, tag="xpad")
                    nc.vector.memset(xpad[:], 0.0)
                    xrp = persist2.tile([128, NB, CR], F32, tag="xrp")
                    p_acc = small.tile([CR, 8], F32, tag="p_acc")
                    for half in range(4):
                        xb = [early.tile([128, 1024], BF16, tag=f"xb{i}",
                                         name=f"xb{s}_{half}_{i}")
                              for i in range(4)]
                        for cc in range(4):
                            nc.gpsimd.dma_start(
                                out=xb[cc][:],
                                in_=x_in.ap()[s, 128 * cc:128 * (cc + 1),
                                              1024 * half:1024 * (half + 1)])
                        for n4 in range(2):
                            n = 2 * half + n4
                            ps = pps.tile([CR, 512], F32, tag="ps")
                            for cc in range(4):
                                nc.tensor.matmul(
                                    ps[:], wredT[:, cc, :],
                                    xb[cc][:, 512 * n4:512 * (n4 + 1)],
                                    start=(cc == 0), stop=(cc == 3))
                            xrstage = early.tile([CR, 512], F32, tag="xrstage",
                                                 name=f"xrs{s}_{n}")
                            nc.vector.tensor_copy(out=xrstage[:], in_=ps[:])
                            # zero-padded interior (rows 8n..8n+8)
                            dst = bass.AP(
                                tensor=xpad.tensor,
                                offset=xpad.offset + (PADW + 8 * n) * PW + PADW,
                                ap=[[xpad.ap[0][0], CR], [PW, 8], [1, W]])
                            nc.scalar.copy(
                                out=dst,
                                in_=ps[:].rearrange("c (r w) -> c r w", r=8))
                            nc.vector.tensor_reduce(
                                out=p_acc[:, n:n + 1], in_=xrstage[:],
                                axis=mybir.AxisListType.X, op=OP.add)
                            for j in range(4):
                                ptr = pps.tile([128, CR], F32, tag="ps",
                                               name=f"ptr{s}_{n}_{j}")
                                nc.tensor.transpose(
                                    ptr[:], xrstage[:, 128 * j:128 * (j + 1)],
                                    idn_f[0:CR, 0:CR])
                                nc.scalar.copy(out=xrp[:, 4 * n + j, :], in_=ptr[:])
                    xrp_all.append(xrp)
                    nc.sync.dma_start(out=dbg_xr.ap()[s],
                                      in_=xrp[:].rearrange("p b c -> p (b c)"))
                    p_sum = small.tile([CR, 1], F32, tag="p_sum")
                    nc.vector.tensor_reduce(out=p_sum[:], in_=p_acc[:],
                                            axis=mybir.AxisListType.X, op=OP.add)

                    # ---- S bases -> DRAM table ----
                    # chunked: basis chunk [28,128] built on the fly, transposed,
                    # staged, one strided DMA per basis.
                    Dy = early.tile([CR, PPR + 4], BF16, tag="Dy")
                    nc.vector.memset(Dy[:], 0.0)
                    ny = PP - PW
                    nc.vector.tensor_tensor(out=Dy[:, 0:ny], in0=xpad[:, PW:PP],
                                            in1=xpad[:, 0:ny], op=OP.subtract)

                    def emit_basis(nm, col0, chunk_fn):
                        stg = early.tile([128, 56, CR], BF16, tag="stg",
                                         name=f"stg_{nm}")
                        for q in range(PPR // 128):
                            srcap = chunk_fn(q)
                            pt = pps.tile([128, CR], BF16, tag="pst",
                                          name=f"pt_{nm}_{q}")
                            nc.tensor.transpose(pt[:], srcap, idn_b[0:CR, 0:CR])
                            nc.scalar.copy(out=stg[:, q, :], in_=pt[:])
                        dstap = bass.AP(
                            tensor=table.ap().tensor,
                            offset=table.ap().offset + s * PPR * EL + col0,
                            ap=[[EL, 128], [128 * EL, 56], [1, CR]])
                        nc.sync.dma_start(out=dstap, in_=stg[:])

                    emit_basis("v", 0, lambda q: xpad[:, 128 * q:128 * (q + 1)])
                    emit_basis("dy", CR, lambda q: Dy[:, 128 * q:128 * (q + 1)])

                    def mk_diff(nm, a, ash, b, bsh):
                        def fn(q):
                            chk = early.tile([CR, 128], BF16, tag="chk",
                                             name=f"chk_{nm}_{q}", bufs=2)
                            nc.vector.tensor_tensor(
                                out=chk[:],
                                in0=a[:, 128 * q + ash:128 * (q + 1) + ash],
                                in1=b[:, 128 * q + bsh:128 * (q + 1) + bsh],
                                op=OP.subtract)
                            return chk[:]
                        return fn
                    emit_basis("dx", 2 * CR, mk_diff("dx", xpad, 1, xpad, 0))
                    emit_basis("dxy", 3 * CR, mk_diff("dxy", Dy, 1, Dy, 0))
                    ctx_e.__exit__(None, None, None)

                    # ---- off conv + transpose to pixel-major ----
                    offT = mid.tile([128, NB, 2 * KT], F32, tag="offT")
                    for n in range(8):
                        po = pps.tile([2 * KT, 512], F32, tag="ps",
                                      name=f"po{s}_{n}")
                        for k in range(KT):
                            kh, kw = k // K, k % K
                            rhs = bass.AP(
                                tensor=xpad.tensor,
                                offset=xpad.offset + (PADW - PAD + kh + 8 * n) * PW
                                       + (PADW - PAD + kw),
                                ap=[[xpad.ap[0][0], CR], [PW, 8], [1, W]])
                            nc.tensor.matmul(po[:], woff_t[:, k, :], rhs,
                                             start=(k == 0), stop=(k == KT - 1))
                        osb = mid.tile([2 * KT, 512], F32, tag="osb",
                                       name=f"osb{s}_{n}")
                        nc.vector.tensor_copy(out=osb[:], in_=po[:])
                        for j in range(4):
                            ptr2 = pps.tile([128, 2 * KT], F32, tag="ps",
                                            name=f"ptr2{s}_{n}_{j}")
                            nc.tensor.transpose(ptr2[:],
                                                osb[:, 128 * j:128 * (j + 1)],
                                                idn_f[0:2 * KT, 0:2 * KT])
                            nc.scalar.copy(out=offT[:, 4 * n + j, :], in_=ptr2[:])
                    ctx_p.__exit__(None, None, None)
                    nc.sync.dma_start(out=dbg_off.ap()[s],
                                      in_=offT[:].rearrange("p b c -> p (b c)"))

                    # ---- frac / index pipeline (pixel-major) ----
                    NF = NB * KT
                    offy = bass.AP(tensor=offT.tensor, offset=offT.offset,
                                   ap=[[offT.ap[0][0], 128], [2 * KT, NB], [2, KT]])
                    offx = bass.AP(tensor=offT.tensor, offset=offT.offset + 1,
                                   ap=[[offT.ap[0][0], 128], [2 * KT, NB], [2, KT]])
                    t_y = mid.tile([128, NB, KT], F32, tag="f1")
                    t_x = mid.tile([128, NB, KT], F32, tag="f2")
                    nc.vector.tensor_tensor(out=t_y[:], in0=offy, in1=grid_y[:],
                                            op=OP.add)
                    nc.vector.tensor_tensor(out=t_x[:], in0=offx, in1=grid_x[:],
                                            op=OP.add)
                    y0i = mid.tile([128, NB, KT], I16, tag="i1")
                    x0i = mid.tile([128, NB, KT], I16, tag="i2")
                    nc.vector.tensor_copy(out=y0i[:], in_=t_y[:])  # round-nearest
                    nc.vector.tensor_copy(out=x0i[:], in_=t_x[:])
                    uy, ux = t_y, t_x   # computed in place
                    nc.vector.scalar_tensor_tensor(
                        out=uy[:], in0=t_y[:], scalar=0.5, in1=y0i[:],
                        op0=OP.add, op1=OP.subtract)
                    nc.vector.scalar_tensor_tensor(
                        out=ux[:], in0=t_x[:], scalar=0.5, in1=x0i[:],
                        op0=OP.add, op1=OP.subtract)
                    uyb = persist.tile([128, NB, KT], BF16, tag="uyb",
                                       name=f"uyb{s}")
                    uxb = persist.tile([128, NB, KT], BF16, tag="uxb",
                                       name=f"uxb{s}")
                    uxyb = persist.tile([128, NB, KT], BF16, tag="uxyb",
                                        name=f"uxyb{s}")
                    nc.vector.tensor_copy(out=uyb[:], in_=uy[:])
                    nc.sync.dma_start(out=dbg_uy.ap()[s],
                                      in_=uy[:].rearrange("p b c -> p (b c)"))
                    nc.vector.tensor_copy(out=uxb[:], in_=ux[:])
                    nc.sync.dma_start(out=dbg_ux.ap()[s],
                                      in_=uxb[:].rearrange("p b c -> p (b c)"))
                    nc.vector.tensor_tensor(out=uy[:], in0=uy[:], in1=ux[:],
                                            op=OP.mult)
                    nc.vector.tensor_copy(out=uxyb[:], in_=uy[:])
                    ctx_m2 = contextlib.ExitStack()
                    ctx_m2.__enter__()
                    midb = ctx_m2.enter_context(
                        tc.tile_pool(name=f"midb{s}", bufs=1))
                    q0i = midb.tile([128, NB, KT], I16, tag="i3")
                    nc.vector.tensor_scalar(out=q0i[:], in0=y0i[:], scalar1=PW,
                                            scalar2=None, op0=OP.mult)
                    nc.vector.tensor_tensor(out=q0i[:], in0=q0i[:], in1=x0i[:],
                                            op=OP.add)

                    # ---- idx fold via DRAM round-trip ----
                    d1 = bass.AP(tensor=q0_dram.ap().tensor,
                                 offset=q0_dram.ap().offset + s * 128 * NF,
                                 ap=[[NF, 8], [8 * NF, 16], [KT, NB], [1, KT]])
                    nc.sync.dma_start(out=d1, in_=q0i[:])
                    nc.sync.dma_start(out=dbg_q0.ap()[s],
                                      in_=q0i[:].rearrange("p b c -> p (b c)"))
                    idx_raw = midb.tile([128, 8, NB, KT], I16, tag="idx_raw")
                    d2src = bass.AP(tensor=q0_dram.ap().tensor,
                                    offset=q0_dram.ap().offset + s * 128 * NF,
                                    ap=[[0, 8], [8 * NF, 16], [1, 8 * NF]])
                    nc.sync.dma_start(
                        out=idx_raw[:].rearrange("p a b k -> p (a b k)"),
                        in_=d2src)
                    idx_w = persist.tile([128, KT, NCH, 64], I16, tag="idx_w",
                                         name=f"idxw{s}")
                    for ch in range(NCH):
                        dst = bass.AP(tensor=idx_w.tensor,
                                      offset=idx_w.offset + ch * 64,
                                      ap=[[idx_w.ap[0][0], 128], [NCH * 64, KT],
                                          [8, 8], [1, 8]])
                        srcap = bass.AP(tensor=idx_raw.tensor,
                                        offset=idx_raw.offset + (8 * ch) * KT,
                                        ap=[[idx_raw.ap[0][0], 128], [1, KT],
                                            [KT, 8], [NB * KT, 8]])
                        nc.vector.tensor_copy(out=dst, in_=srcap)
                    ctx_m2.__exit__(None, None, None)

                # ---- gather + combine + tap-contraction ----
                # gathers rotate over 4 SWDGE queues; an all-engine barrier per
                # group makes the cumulative DMA-sem waits order-safe, while
                # group g+1 gathers overlap group-g combines.
                psaT = ppa.tile([128, NB, CR], F32, tag="acc", name=f"psaT{s}")
                qn = 0
                G4s = {}

                def do_gathers(g):
                    nonlocal qn
                    for kk in range(GSZ):
                        k = GSZ * g + kk
                        if k >= KT:
                            continue
                        G4 = gpool.tile([128, NB, EL], BF16, tag="G4", bufs=4,
                                        name=f"G4_{s}_{k}")
                        for ch in range(NCH):
                            nc.gpsimd.dma_gather(
                                out_ap=G4[:, 8 * ch:8 * (ch + 1), :],
                                in_ap=table.ap()[s],
                                idxs_ap=idx_w[:, k, ch, :],
                                num_idxs=1024, num_idxs_reg=1024, elem_size=EL,
                                queue_num=0)
                            qn += 1
                        G4s[k] = G4

                do_gathers(0)
                for g in range(NG):
                    tc.strict_bb_all_engine_barrier()
                    if g + 1 < NG:
                        do_gathers(g + 1)
                    phi = gpool.tile([128, NB, GSZ, CPAD], BF16, tag="phi",
                                     name=f"phi{s}_{g}")
                    nc.vector.memset(phi[:], 0.0)
                    for kk in range(GSZ):
                        k = GSZ * g + kk
                        if k >= KT:
                            continue
                        G4 = G4s.pop(k)
                        pslc = phi[:, :, kk, 0:CR]
                        nc.scalar.copy(out=pslc, in_=G4[:, :, 0:CR])
                        for (bi, ut) in ((1, uyb), (2, uxb), (3, uxyb)):
                            ub = bass.AP(tensor=ut.tensor, offset=ut.offset + k,
                                         ap=[[ut.ap[0][0], 128], [KT, NB],
                                             [0, CR]])
                            tt = gpool.tile([128, NB, CR], BF16, tag="tt",
                                            name=f"tt{s}_{k}_{bi}")
                            nc.vector.tensor_tensor(
                                out=tt[:], in0=G4[:, :, CR * bi:CR * (bi + 1)],
                                in1=ub, op=OP.mult)
                            nc.vector.tensor_tensor(out=pslc, in0=pslc,
                                                    in1=tt[:], op=OP.add)
                    if g == 0:
                        nc.sync.dma_start(
                            out=dbg_phi.ap()[s],
                            in_=phi[:].rearrange("p b t c -> p (b t c)"))
                    for b in range(NB):
                        ptp = pps.tile([GSZ * CPAD, 128], BF16, tag="pst",
                                       name=f"ptp{s}_{g}_{b}")
                        nc.tensor.transpose(ptp[:], phi[:, b, :, :], idn_b[:])
                        phT = small.tile([GSZ * CPAD, 128], BF16, tag="phT",
                                         bufs=3, name=f"phT{s}_{g}_{b}")
                        nc.scalar.copy(out=phT[:], in_=ptp[:])
                        nc.tensor.matmul(psaT[:, b, :], phT[:], wdT[:, g, :],
                                         start=(g == 0), stop=(g == NG - 1))

                saT = persist2.tile([128, NB, CR], F32, tag="saT",
                                    name=f"saT{s}")
                nc.vector.tensor_copy(out=saT[:], in_=psaT[:])
                saT_all.append(saT)
                nc.sync.dma_start(out=dbg_sa.ap()[s],
                                  in_=saT[:].rearrange("p b c -> p (b c)"))

                # ---- SE gate ----
                cmax = small.tile([128, NB], F32, tag="cmax")
                nc.vector.tensor_reduce(out=cmax[:], in_=xrp[:],
                                        axis=mybir.AxisListType.X, op=OP.max)
                cm1 = small.tile([128, 1], F32, tag="cm1")
                nc.vector.tensor_reduce(out=cm1[:], in_=cmax[:],
                                        axis=mybir.AxisListType.X, op=OP.add)
                pchi = pps.tile([1, 1], F32, tag="ps", name=f"pchi{s}")
                nc.tensor.matmul(pchi[:], cm1[:], ones128[:], start=True,
                                 stop=True)
                chi = small.tile([1, 1], F32, tag="chi")
                nc.vector.tensor_copy(out=chi[:], in_=pchi[:])
                mlp_in = small.tile([CR, 2], BF16, tag="mlp_in")
                nc.scalar.activation(out=mlp_in[:, 0:1], in_=p_sum[:],
                                     func=AF.Copy, scale=1.0 / P)
                pbc = pps.tile([CR, 1], F32, tag="ps", name=f"pbc{s}")
                nc.tensor.matmul(pbc[:], ones_r[:, 0:CR], chi[:], start=True,
                                 stop=True)
                nc.scalar.activation(out=mlp_in[:, 1:2], in_=pbc[:], func=AF.Copy,
                                     scale=1.0 / P)
                ph1 = pps.tile([CR, 2], F32, tag="ps", name=f"ph1{s}")
                nc.tensor.matmul(ph1[:], wse1T[:], mlp_in[:], start=True,
                                 stop=True)
                h1 = small.tile([CR, 2], BF16, tag="h1")
                nc.scalar.activation(out=h1[:], in_=ph1[:], func=AF.Relu,
                                     bias=bse1[:])
                ph2 = pps.tile([CR, 2], F32, tag="ps", name=f"ph2{s}")
                nc.tensor.matmul(ph2[:], wse2T[:], h1[:], start=True, stop=True)
                se2 = small.tile([CR, 2], F32, tag="se2")
                nc.scalar.activation(out=se2[:], in_=ph2[:], func=AF.Sigmoid,
                                     bias=bse2[:])
                se_v = small.tile([CR, 1], F32, tag="se_v")
                nc.vector.tensor_tensor(out=se_v[:], in0=se2[:, 0:1],
                                        in1=se2[:, 1:2], op=OP.add)
                prow = pps.tile([1, CR], F32, tag="ps", name=f"prow{s}")
                nc.tensor.matmul(prow[:], se_v[:], idn_f[0:CR, 0:CR],
                                 start=True, stop=True)
                se_row = persist2.tile([1, CR], F32, tag="se_row",
                                       name=f"serow{s}")
                nc.vector.tensor_scalar(out=se_row[:], in0=prow[:],
                                        scalar1=float(wse_f), scalar2=None,
                                        op0=OP.mult)
                se_all.append(se_row)

                # ---- BN partial stats ----
                sq = small.tile([128, NB * CR], F32, tag="sq")
                nc.scalar.activation(
                    out=sq[:], in_=saT[:].rearrange("p b c -> p (b c)"),
                    func=AF.Square)
                s_sum = small.tile([1, NB * CR], F32, tag="s_sum")
                s_sq = small.tile([1, NB * CR], F32, tag="s_sq")
                for hh in range(2):
                    sl = slice(448 * hh, 448 * (hh + 1))
                    pst = pps.tile([1, 448], F32, tag="ps", name=f"pst{s}_{hh}")
                    nc.tensor.matmul(
                        pst[:], ones128[:],
                        saT[:].rearrange("p b c -> p (b c)")[:, sl],
                        start=True, stop=True)
                    nc.vector.tensor_copy(out=s_sum[:, sl], in_=pst[:])
                    pst2 = pps.tile([1, 448], F32, tag="ps", name=f"ps2{s}_{hh}")
                    nc.tensor.matmul(pst2[:], ones128[:], sq[:, sl],
                                     start=True, stop=True)
                    nc.vector.tensor_copy(out=s_sq[:, sl], in_=pst2[:])

                # fold over b -> [1, CR]
                def fold_b(t, nm):
                    cur, n = t, NB
                    while n > 1:
                        h = n // 2
                        nxt = small.tile([1, h, CR], F32, tag="foldb",
                                         name=f"fb_{nm}_{h}", bufs=2)
                        nc.vector.tensor_tensor(
                            out=nxt[:],
                            in0=bass.AP(tensor=cur.tensor, offset=cur.offset,
                                        ap=[[cur.ap[0][0], 1], [CR, h], [1, CR]]),
                            in1=bass.AP(tensor=cur.tensor,
                                        offset=cur.offset + h * CR,
                                        ap=[[cur.ap[0][0], 1], [CR, h], [1, CR]]),
                            op=OP.add)
                        cur, n = nxt, h
                    return cur
                f_sum = fold_b(s_sum, f"su{s}")
                f_sq = fold_b(s_sq, f"sq{s}")
                st_t = persist2.tile([1, 2, CR], F32, tag="st_t",
                                     name=f"stt{s}")
                nc.vector.tensor_copy(out=st_t[:, 0, :], in_=f_sum[:, 0, :])
                nc.vector.tensor_copy(out=st_t[:, 1, :], in_=f_sq[:, 0, :])
                stat_s.append(st_t)

            # ---- global BN stats (AllReduce) ----
            stat = small.tile([1, 64], F32, tag="stat")
            nc.vector.memset(stat[:], 0.0)
            nc.vector.tensor_tensor(out=stat[:, 0:CR], in0=stat_s[0][:, 0, :],
                                    in1=stat_s[1][:, 0, :], op=OP.add)
            nc.vector.tensor_tensor(out=stat[:, 32:32 + CR],
                                    in0=stat_s[0][:, 1, :],
                                    in1=stat_s[1][:, 1, :], op=OP.add)
            nc.sync.dma_start(out=ccin.ap(), in_=stat[:])
            nc.gpsimd.collective_compute(
                "AllReduce", OP.add, replica_groups=groups,
                ins=[ccin.ap()], outs=[ccout.ap()])
            statg = small.tile([1, 64], F32, tag="statg")
            nc.sync.dma_start(out=statg[:], in_=ccout.ap())

            Nt = float(B * P)
            mu = small.tile([1, CR], F32, tag="mu")
            nc.scalar.activation(out=mu[:], in_=statg[:, 0:CR], func=AF.Copy,
                                 scale=1.0 / Nt)
            e2 = small.tile([1, CR], F32, tag="e2")
            nc.scalar.activation(out=e2[:], in_=statg[:, 32:32 + CR],
                                 func=AF.Copy, scale=1.0 / Nt)
            mu2 = small.tile([1, CR], F32, tag="mu2")
            nc.vector.tensor_tensor(out=mu2[:], in0=mu[:], in1=mu[:], op=OP.mult)
            var = small.tile([1, CR], F32, tag="var")
            nc.vector.tensor_tensor(out=var[:], in0=e2[:], in1=mu2[:],
                                    op=OP.subtract)
            sd = small.tile([1, CR], F32, tag="sd")
            nc.vector.tensor_scalar(out=sd[:], in0=var[:], scalar1=1e-5,
                                    scalar2=None, op0=OP.add)
            nc.scalar.sqrt(out=sd[:], in_=sd[:])
            rsd = small.tile([1, CR], F32, tag="rsd")
            nc.vector.reciprocal(out=rsd[:], in_=sd[:])
            s_row = small.tile([1, CR], F32, tag="s_row")
            nc.vector.tensor_tensor(out=s_row[:], in0=gamma_r[:], in1=rsd[:],
                                    op=OP.mult)
            t_row = small.tile([1, CR], F32, tag="t_row")
            nc.vector.tensor_tensor(out=t_row[:], in0=mu[:], in1=s_row[:],
                                    op=OP.mult)
            nc.vector.tensor_tensor(out=t_row[:], in0=beta_r[:], in1=t_row[:],
                                    op=OP.subtract)

            def repl(row, tag, nm):
                pr = pps.tile([128, CR], F32, tag="ps", name=f"pr_{nm}")
                nc.tensor.matmul(pr[:], ones_r[:], row[:], start=True, stop=True)
                t = small.tile([128, CR], F32, tag=tag, name=f"rep_{nm}")
                nc.vector.tensor_copy(out=t[:], in_=pr[:])
                return t
            s_rep = repl(s_row, "s_rep", "s")
            t_rep = repl(t_row, "t_rep", "t")

            # ---- phase 2: BN apply, mix, restore ----
            for s in range(SLOC):
                ctx2 = contextlib.ExitStack()
                with ctx2:
                    ph2p = ctx2.enter_context(
                        tc.tile_pool(name=f"ph2_{s}", bufs=1))
                    saT = saT_all[s]
                    xrp = xrp_all[s]
                    se_rep = repl(se_all[s], "se_rep", f"se{s}")
                    a1 = ph2p.tile([128, NB, CR], F32, tag="a1")
                    srep_b = bass.AP(tensor=s_rep.tensor, offset=s_rep.offset,
                                     ap=[[s_rep.ap[0][0], 128], [0, NB], [1, CR]])
                    trep_b = bass.AP(tensor=t_rep.tensor, offset=t_rep.offset,
                                     ap=[[t_rep.ap[0][0], 128], [0, NB], [1, CR]])
                    nc.vector.tensor_tensor(out=a1[:], in0=saT[:], in1=srep_b,
                                            op=OP.mult)
                    nc.vector.tensor_tensor(out=a1[:], in0=a1[:], in1=trep_b,
                                            op=OP.add)
                    af = a1[:].rearrange("p b c -> p (b c)")
                    nc.scalar.activation(out=af, in_=af, func=AF.Relu)
                    nc.scalar.activation(out=af, in_=af, func=AF.Sigmoid)
                    u = ph2p.tile([128, NB, CR], F32, tag="u")
                    nc.vector.tensor_scalar(out=u[:], in0=a1[:],
                                            scalar1=float(wsa_f), scalar2=None,
                                            op0=OP.mult)
                    serep_b = bass.AP(tensor=se_rep.tensor,
                                      offset=se_rep.offset,
                                      ap=[[se_rep.ap[0][0], 128], [0, NB],
                                          [1, CR]])
                    nc.vector.tensor_tensor(out=u[:], in0=u[:], in1=serep_b,
                                            op=OP.add)
                    nc.vector.tensor_tensor(out=u[:], in0=u[:], in1=xrp[:],
                                            op=OP.mult)
                    mix_ch = ph2p.tile([CR, P], BF16, tag="mix_ch")
                    for b in range(NB):
                        pmt = pps.tile([CR, 128], F32, tag="ps",
                                       name=f"pmt{s}_{b}")
                        nc.tensor.transpose(pmt[:], u[:, b, :], idn_f[:])
                        nc.scalar.copy(out=mix_ch[:, 128 * b:128 * (b + 1)],
                                       in_=pmt[:])
                    for ccc in range(4):
                        for n in range(8):
                            pr = pps.tile([128, 512], F32, tag="ps",
                                          name=f"prr{s}_{ccc}_{n}")
                            nc.tensor.matmul(pr[:], wresT[:, ccc, :],
                                             mix_ch[:, 512 * n:512 * (n + 1)],
                                             start=True, stop=True)
                            ot = ph2p.tile([128, 512], F32, tag="ot", bufs=3,
                                           name=f"ot{s}_{ccc}_{n}")
                            nc.vector.tensor_copy(out=ot[:], in_=pr[:])
                            nc.sync.dma_start(
                                out=out_d.ap()[s, 128 * ccc:128 * (ccc + 1),
                                               512 * n:512 * (n + 1)],
                                in_=ot[:])
    nc.compile()
    return nc


def kernel(**inputs):
    x = np.asarray(inputs["x"], np.float32).reshape(B, C, P)
    consts = host_consts(
        np.asarray(inputs["w_red"], np.float32),
        np.asarray(inputs["w_se1"], np.float32),
        np.asarray(inputs["b_se1"], np.float32),
        np.asarray(inputs["w_se2"], np.float32),
        np.asarray(inputs["b_se2"], np.float32),
        np.asarray(inputs["w_off"], np.float32),
        np.asarray(inputs["b_off"], np.float32),
        np.asarray(inputs["w_dcn"], np.float32),
        np.asarray(inputs["bn_gamma"], np.float32),
        np.asarray(inputs["bn_beta"], np.float32),
        np.asarray(inputs["w_res"], np.float32),
        np.asarray(inputs["weight_se"], np.float32),
        np.asarray(inputs["weight_sa"], np.float32),
    )
    key = ("nc", consts["wse_f"], consts["wsa_f"])
    if key not in _cache:
        _cache[key] = build(local_bn=False, wse_f=consts["wse_f"],
                            wsa_f=consts["wsa_f"])
        _cache["nc"] = _cache[key]
    nc = _cache[key]
    base = {k: v for k, v in consts.items() if not isinstance(v, float)}
    in_maps = []
    for i in range(NCORES):
        m = dict(base)
        m["x"] = np.ascontiguousarray(x[SLOC * i:SLOC * (i + 1)])
        in_maps.append(m)
    res = run_bass_kernel_spmd(nc, in_maps, core_ids=list(range(NCORES)),
                               trace=False)
    out = np.concatenate([res.results[i]["out"] for i in range(NCORES)], axis=0)
    return out.reshape(B, C, H, W)



# revision 4
# speedup vs baseline: 4.0915x; 4.0915x over previous
"""Trainium2 Bass kernel for nn_ADAM_17575006175639 (deformable-conv attention block).

Strategy (8 NeuronCores, data-parallel over batch, 2 samples/core):
  - xr = w_red @ x on PE (bf16)
  - offset conv 7x7 on PE (49 shifted K=28 matmuls over zero-padded xr)
  - deformable bilinear sampling: per-(tap,pixel) row gather from a DRAM table
    [v | Dy | Dx | Dxy] (difference basis) via gpsimd dma_gather (4 SWDGE
    queues), landing pixel-major so tent weights apply as per-pixel broadcasts
    on DVE; tap-contraction (w_dcn) on PE via per-group PE transposes
  - BN stats all-reduced across the 8 cores (tiny collective)
  - SE gate + mix + w_res restore on PE/ACT/DVE
"""
import os
import numpy as np
import ml_dtypes

import concourse.bass as bass
import concourse.bacc as bacc
import concourse.tile as tile
from concourse import mybir
from concourse.bass_utils import run_bass_kernel_spmd

F32 = mybir.dt.float32
BF16 = mybir.dt.bfloat16
I16 = mybir.dt.int16
AF = mybir.ActivationFunctionType
OP = mybir.AluOpType

B, C, H, W = 16, 512, 64, 64
CR, K, PAD = 28, 7, 3
NCORES = 8
SLOC = B // NCORES            # samples per core
P = H * W                     # 4096 pixels
PADW = 10                     # zero-pad margin (|offset| must stay < 7)
PW = H + 2 * PADW             # 84 padded width
PP = PW * PW                  # 7056 padded pixels
PPR = 7168                    # padded to 56*128 for transpose chunks
NB = P // 128                 # 32 pixel blocks
KT = K * K                    # 49 taps
NG = 25                       # tap groups of 2 (50 >= 49)
GSZ = 2                       # taps per group
CPAD = 32                     # padded channel dim in Phi
NCH = 4                       # gather chunks per tap (1024 idx each)
EL = 128                      # table row elements (bf16) = 256B
WSE, WSA = None, None         # folded on host

_cache = {}


def _to_bf(a):
    return np.ascontiguousarray(a.astype(ml_dtypes.bfloat16))


def host_consts(w_red, w_se1, b_se1, w_se2, b_se2, w_off, b_off, w_dcn,
                bn_gamma, bn_beta, w_res, weight_se, weight_sa):
    c = {}
    c["wredT"] = _to_bf(w_red.T.reshape(4, 128, CR))                  # [4,128,28]
    c["woffT"] = _to_bf(np.transpose(w_off, (2, 3, 1, 0)).reshape(KT, CR, 2 * KT)
                        .transpose(1, 0, 2))  # [28,49,98]
    # wd rows: group g, kk in 0..GSZ-1 -> tap k=GSZ*g+kk, channel c' (pad zero)
    wd = np.zeros((NG, GSZ, CPAD, CR), np.float32)
    for g in range(NG):
        for kk in range(GSZ):
            k = GSZ * g + kk
            if k < KT:
                kh, kw = k // K, k % K
                wd[g, kk, :CR, :] = w_dcn[:, :, kh, kw].T             # [c_in, o]
    c["wdT"] = _to_bf(wd.reshape(NG, GSZ * CPAD, CR))                 # [25,64,28]
    c["wresT"] = _to_bf(w_res.T.reshape(CR, 4, 128))                  # [28,4,128]
    c["wse1T"] = _to_bf(w_se1.T)                                      # [28,28]
    c["wse2T"] = _to_bf(w_se2.T)
    c["bse1"] = np.ascontiguousarray(b_se1.reshape(CR, 1).astype(np.float32))
    c["bse2"] = np.ascontiguousarray(b_se2.reshape(CR, 1).astype(np.float32))
    c["gamma_r"] = np.ascontiguousarray(bn_gamma.reshape(1, CR).astype(np.float32))
    c["beta_r"] = np.ascontiguousarray(bn_beta.reshape(1, CR).astype(np.float32))
    c["idn_b"] = _to_bf(np.eye(128, dtype=np.float32))
    c["idn_f"] = np.eye(128, dtype=np.float32)
    c["ones128"] = np.ones((128, 1), np.float32)
    c["ones_r"] = np.ones((1, 128), np.float32)
    # grids: pixel pi = 128*b + p ; i = pi//64, j = pi%64
    p_idx = np.arange(128)
    b_idx = np.arange(NB)
    pi = (128 * b_idx[None, :, None] + p_idx[:, None, None])          # [128,32,1]
    ii = pi // W
    jj = pi % W
    kh = (np.arange(KT) // K)[None, None, :]
    kw = (np.arange(KT) % K)[None, None, :]
    gy = ii + kh + (PADW - PAD) + b_off[0::2][None, None, :] - 0.5
    gx = jj + kw + (PADW - PAD) + b_off[1::2][None, None, :] - 0.5
    c["grid_y"] = _to_bf(gy)          # [128,32,49] (exactly representable)
    c["grid_x"] = _to_bf(gx)
    assert np.all(c["grid_y"].astype(np.float64) == gy)
    assert np.all(c["grid_x"].astype(np.float64) == gx)
    wse = float(weight_se.reshape(-1)[0])
    wsa = float(weight_sa.reshape(-1)[0])
    den = wse + wsa + 1e-6
    c["wse_f"] = wse / den
    c["wsa_f"] = wsa / den
    return c


def build(local_bn=False, wse_f=0.4999995, wsa_f=0.4999995):
    """Builds the SPMD program. local_bn: replica groups of 1 (for sim)."""
    nc = bacc.Bacc("TRN2", target_bir_lowering=False, debug=False,
                   num_devices=NCORES, num_swdge_queues=4)
    x_in = nc.dram_tensor("x", [SLOC, C, P], F32, kind="ExternalInput")
    cN = {}
    for nm, shape, dt in [
        ("wredT", [4, 128, CR], BF16), ("woffT", [CR, KT, 2 * KT], BF16),
        ("wdT", [NG, GSZ * CPAD, CR], BF16), ("wresT", [CR, 4, 128], BF16),
        ("wse1T", [CR, CR], BF16), ("wse2T", [CR, CR], BF16),
        ("bse1", [CR, 1], F32), ("bse2", [CR, 1], F32),
        ("gamma_r", [1, CR], F32), ("beta_r", [1, CR], F32),
        ("idn_b", [128, 128], BF16), ("idn_f", [128, 128], F32),
        ("ones128", [128, 1], F32), ("ones_r", [1, 128], F32),
        ("grid_y", [128, NB, KT], BF16), ("grid_x", [128, NB, KT], BF16),
    ]:
        cN[nm] = nc.dram_tensor(nm, shape, dt, kind="ExternalInput")
    out_d = nc.dram_tensor("out", [SLOC, C, P], F32, kind="ExternalOutput")
    dbg_xr = nc.dram_tensor("dbg_xr", [SLOC, 128, NB * CR], F32, kind="ExternalOutput")
    dbg_off = nc.dram_tensor("dbg_off", [SLOC, 128, NB * 2 * KT], F32, kind="ExternalOutput")
    dbg_sa = nc.dram_tensor("dbg_sa", [SLOC, 128, NB * CR], F32, kind="ExternalOutput")
    dbg_uy = nc.dram_tensor("dbg_uy", [SLOC, 128, NB * KT], F32, kind="ExternalOutput")
    dbg_q0 = nc.dram_tensor("dbg_q0", [SLOC, 128, NB * KT], I16, kind="ExternalOutput")
    dbg_phi = nc.dram_tensor("dbg_phi", [SLOC, 128, NB * GSZ * CPAD], BF16, kind="ExternalOutput")
    dbg_ux = nc.dram_tensor("dbg_ux", [SLOC, 128, NB * KT], BF16, kind="ExternalOutput")
    table = nc.dram_tensor("tbl", [SLOC, PPR, EL], BF16, kind="ExternalOutput")
    q0_dram = nc.dram_tensor("q0scr", [SLOC, 128 * NB * KT], I16, kind="Internal")
    ccin = nc.dram_tensor("ccin", [1, 64], F32, kind="Internal")
    ccout = nc.dram_tensor("ccout", [1, 64], F32, kind="Internal",
                           addr_space="Shared")
    groups = [[i] for i in range(NCORES)] if local_bn else [list(range(NCORES))]

    with tile.TileContext(nc) as tc:
        import contextlib
        ctx = contextlib.ExitStack()
        with ctx:
            sing = ctx.enter_context(tc.tile_pool(name="sing", bufs=1))
            persist = ctx.enter_context(tc.tile_pool(name="persist", bufs=1))
            persist2 = ctx.enter_context(tc.tile_pool(name="persist2", bufs=2))
            small = ctx.enter_context(tc.tile_pool(name="small", bufs=1))
            gpool = ctx.enter_context(tc.tile_pool(name="g", bufs=2))
            ppa = ctx.enter_context(tc.tile_pool(name="ppa", bufs=2, space="PSUM"))
            pps = ctx.enter_context(tc.tile_pool(name="pps", bufs=2, space="PSUM"))

            # ---- constants to SBUF ----
            wredT = sing.tile([128, 4, CR], BF16)
            nc.sync.dma_start(out=wredT[:],
                              in_=cN["wredT"].ap().rearrange("c p m -> p c m"))
            wdT = sing.tile([GSZ * CPAD, NG, CR], BF16)
            nc.sync.dma_start(out=wdT[:],
                              in_=cN["wdT"].ap().rearrange("g p m -> p g m"))

            def _load(nm, shape, dt):
                t = sing.tile(shape, dt, name=nm + "_t")
                nc.sync.dma_start(out=t[:], in_=cN[nm].ap())
                return t
            woff_t = _load("woffT", [CR, KT, 2 * KT], BF16)
            wresT = _load("wresT", [CR, 4, 128], BF16)
            wse1T = _load("wse1T", [CR, CR], BF16)
            wse2T = _load("wse2T", [CR, CR], BF16)
            bse1 = _load("bse1", [CR, 1], F32)
            bse2 = _load("bse2", [CR, 1], F32)
            gamma_r = _load("gamma_r", [1, CR], F32)
            beta_r = _load("beta_r", [1, CR], F32)
            idn_b = _load("idn_b", [128, 128], BF16)
            idn_f = _load("idn_f", [128, 128], F32)
            ones128 = _load("ones128", [128, 1], F32)
            ones_r = _load("ones_r", [1, 128], F32)
            grid_y = _load("grid_y", [128, NB, KT], BF16)
            grid_x = _load("grid_x", [128, NB, KT], BF16)

            saT_all, xrp_all, se_all, stat_s = [], [], [], []

            for s in range(SLOC):
                ctx_s = contextlib.ExitStack()
                with ctx_s:
                    mid = ctx_s.enter_context(
                        tc.tile_pool(name=f"mid{s}", bufs=1))
                    ctx_p = contextlib.ExitStack()
                    ctx_p.__enter__()
                    pxp = ctx_p.enter_context(
                        tc.tile_pool(name=f"pxp{s}", bufs=1))
                    ctx_e = contextlib.ExitStack()
                    ctx_e.__enter__()
                    early = ctx_e.enter_context(
                        tc.tile_pool(name=f"early{s}", bufs=1))

                    # ---- xr matmul (stream x in halves, cast-DMA f32->bf16) ----
                    xpad = pxp.tile([CR, PPR + 4], BF16, tag="xpad")
                    nc.vector.memset(xpad[:], 0.0)
                    xrp = persist2.tile([128, NB, CR], F32, tag="xrp")
                    p_acc = small.tile([CR, 8], F32, tag="p_acc")
                    for half in range(4):
                        xb = [early.tile([128, 1024], BF16, tag=f"xb{i}",
                                         name=f"xb{s}_{half}_{i}")
                              for i in range(4)]
                        for cc in range(4):
                            nc.gpsimd.dma_start(
                                out=xb[cc][:],
                                in_=x_in.ap()[s, 128 * cc:128 * (cc + 1),
                                              1024 * half:1024 * (half + 1)])
                        for n4 in range(2):
                            n = 2 * half + n4
                            ps = pps.tile([CR, 512], F32, tag="ps")
                            for cc in range(4):
                                nc.tensor.matmul(
                                    ps[:], wredT[:, cc, :],
                                    xb[cc][:, 512 * n4:512 * (n4 + 1)],
                                    start=(cc == 0), stop=(cc == 3))
                            xrstage = early.tile([CR, 512], F32, tag="xrstage",
                                                 name=f"xrs{s}_{n}")
                            nc.vector.tensor_copy(out=xrstage[:], in_=ps[:])
                            # zero-padded interior (rows 8n..8n+8)
                            dst = bass.AP(
                                tensor=xpad.tensor,
                                offset=xpad.offset + (PADW + 8 * n) * PW + PADW,
                                ap=[[xpad.ap[0][0], CR], [PW, 8], [1, W]])
                            nc.scalar.copy(
                                out=dst,
                                in_=ps[:].rearrange("c (r w) -> c r w", r=8))
                            nc.vector.tensor_reduce(
                                out=p_acc[:, n:n + 1], in_=xrstage[:],
                                axis=mybir.AxisListType.X, op=OP.add)
                            for j in range(4):
                                ptr = pps.tile([128, CR], F32, tag="ps",
                                               name=f"ptr{s}_{n}_{j}")
                                nc.tensor.transpose(
                                    ptr[:], xrstage[:, 128 * j:128 * (j + 1)],
                                    idn_f[0:CR, 0:CR])
                                nc.scalar.copy(out=xrp[:, 4 * n + j, :], in_=ptr[:])
                    xrp_all.append(xrp)
                    nc.sync.dma_start(out=dbg_xr.ap()[s],
                                      in_=xrp[:].rearrange("p b c -> p (b c)"))
                    p_sum = small.tile([CR, 1], F32, tag="p_sum")
                    nc.vector.tensor_reduce(out=p_sum[:], in_=p_acc[:],
                                            axis=mybir.AxisListType.X, op=OP.add)

                    # ---- S bases -> DRAM table ----
                    # chunked: basis chunk [28,128] built on the fly, transposed,
                    # staged, one strided DMA per basis.
                    Dy = early.tile([CR, PPR + 4], BF16, tag="Dy")
                    nc.vector.memset(Dy[:], 0.0)
                    ny = PP - PW
                    nc.vector.tensor_tensor(out=Dy[:, 0:ny], in0=xpad[:, PW:PP],
                                            in1=xpad[:, 0:ny], op=OP.subtract)

                    def emit_basis(nm, col0, chunk_fn):
                        stg = early.tile([128, 56, CR], BF16, tag="stg",
                                         name=f"stg_{nm}")
                        for q in range(PPR // 128):
                            srcap = chunk_fn(q)
                            pt = pps.tile([128, CR], BF16, tag="pst",
                                          name=f"pt_{nm}_{q}")
                            nc.tensor.transpose(pt[:], srcap, idn_b[0:CR, 0:CR])
                            nc.scalar.copy(out=stg[:, q, :], in_=pt[:])
                        dstap = bass.AP(
                            tensor=table.ap().tensor,
                            offset=table.ap().offset + s * PPR * EL + col0,
                            ap=[[EL, 128], [128 * EL, 56], [1, CR]])
                        nc.sync.dma_start(out=dstap, in_=stg[:])

                    emit_basis("v", 0, lambda q: xpad[:, 128 * q:128 * (q + 1)])
                    emit_basis("dy", CR, lambda q: Dy[:, 128 * q:128 * (q + 1)])

                    def mk_diff(nm, a, ash, b, bsh):
                        def fn(q):
                            chk = early.tile([CR, 128], BF16, tag="chk",
                                             name=f"chk_{nm}_{q}", bufs=2)
                            nc.vector.tensor_tensor(
                                out=chk[:],
                                in0=a[:, 128 * q + ash:128 * (q + 1) + ash],
                                in1=b[:, 128 * q + bsh:128 * (q + 1) + bsh],
                                op=OP.subtract)
                            return chk[:]
                        return fn
                    emit_basis("dx", 2 * CR, mk_diff("dx", xpad, 1, xpad, 0))
                    emit_basis("dxy", 3 * CR, mk_diff("dxy", Dy, 1, Dy, 0))
                    ctx_e.__exit__(None, None, None)

                    # ---- off conv + transpose to pixel-major ----
                    offT = mid.tile([128, NB, 2 * KT], F32, tag="offT")
                    for n in range(8):
                        po = pps.tile([2 * KT, 512], F32, tag="ps",
                                      name=f"po{s}_{n}")
                        for k in range(KT):
                            kh, kw = k // K, k % K
                            rhs = bass.AP(
                                tensor=xpad.tensor,
                                offset=xpad.offset + (PADW - PAD + kh + 8 * n) * PW
                                       + (PADW - PAD + kw),
                                ap=[[xpad.ap[0][0], CR], [PW, 8], [1, W]])
                            nc.tensor.matmul(po[:], woff_t[:, k, :], rhs,
                                             start=(k == 0), stop=(k == KT - 1))
                        osb = mid.tile([2 * KT, 512], F32, tag="osb",
                                       name=f"osb{s}_{n}")
                        nc.vector.tensor_copy(out=osb[:], in_=po[:])
                        for j in range(4):
                            ptr2 = pps.tile([128, 2 * KT], F32, tag="ps",
                                            name=f"ptr2{s}_{n}_{j}")
                            nc.tensor.transpose(ptr2[:],
                                                osb[:, 128 * j:128 * (j + 1)],
                                                idn_f[0:2 * KT, 0:2 * KT])
                            nc.scalar.copy(out=offT[:, 4 * n + j, :], in_=ptr2[:])
                    ctx_p.__exit__(None, None, None)
                    nc.sync.dma_start(out=dbg_off.ap()[s],
                                      in_=offT[:].rearrange("p b c -> p (b c)"))

                    # ---- frac / index pipeline (pixel-major) ----
                    NF = NB * KT
                    offy = bass.AP(tensor=offT.tensor, offset=offT.offset,
                                   ap=[[offT.ap[0][0], 128], [2 * KT, NB], [2, KT]])
                    offx = bass.AP(tensor=offT.tensor, offset=offT.offset + 1,
                                   ap=[[offT.ap[0][0], 128], [2 * KT, NB], [2, KT]])
                    t_y = mid.tile([128, NB, KT], F32, tag="f1")
                    t_x = mid.tile([128, NB, KT], F32, tag="f2")
                    nc.vector.tensor_tensor(out=t_y[:], in0=offy, in1=grid_y[:],
                                            op=OP.add)
                    nc.vector.tensor_tensor(out=t_x[:], in0=offx, in1=grid_x[:],
                                            op=OP.add)
                    y0i = mid.tile([128, NB, KT], I16, tag="i1")
                    x0i = mid.tile([128, NB, KT], I16, tag="i2")
                    nc.vector.tensor_copy(out=y0i[:], in_=t_y[:])  # round-nearest
                    nc.vector.tensor_copy(out=x0i[:], in_=t_x[:])
                    uy, ux = t_y, t_x   # computed in place
                    nc.vector.scalar_tensor_tensor(
                        out=uy[:], in0=t_y[:], scalar=0.5, in1=y0i[:],
                        op0=OP.add, op1=OP.subtract)
                    nc.vector.scalar_tensor_tensor(
                        out=ux[:], in0=t_x[:], scalar=0.5, in1=x0i[:],
                        op0=OP.add, op1=OP.subtract)
                    uyb = persist.tile([128, NB, KT], BF16, tag="uyb",
                                       name=f"uyb{s}")
                    uxb = persist.tile([128, NB, KT], BF16, tag="uxb",
                                       name=f"uxb{s}")
                    uxyb = persist.tile([128, NB, KT], BF16, tag="uxyb",
                                        name=f"uxyb{s}")
                    nc.vector.tensor_copy(out=uyb[:], in_=uy[:])
                    nc.sync.dma_start(out=dbg_uy.ap()[s],
                                      in_=uy[:].rearrange("p b c -> p (b c)"))
                    nc.vector.tensor_copy(out=uxb[:], in_=ux[:])
                    nc.sync.dma_start(out=dbg_ux.ap()[s],
                                      in_=uxb[:].rearrange("p b c -> p (b c)"))
                    nc.vector.tensor_tensor(out=uy[:], in0=uy[:], in1=ux[:],
                                            op=OP.mult)
                    nc.vector.tensor_copy(out=uxyb[:], in_=uy[:])
                    ctx_m2 = contextlib.ExitStack()
                    ctx_m2.__enter__()
                    midb = ctx_m2.enter_context(
                        tc.tile_pool(name=f"midb{s}", bufs=1))
                    q0i = midb.tile([128, NB, KT], I16, tag="i3")
                    nc.vector.tensor_scalar(out=q0i[:], in0=y0i[:], scalar1=PW,
                                            scalar2=None, op0=OP.mult)
                    nc.vector.tensor_tensor(out=q0i[:], in0=q0i[:], in1=x0i[:],
                                            op=OP.add)

                    # ---- idx fold via DRAM round-trip ----
                    d1 = bass.AP(tensor=q0_dram.ap().tensor,
                                 offset=q0_dram.ap().offset + s * 128 * NF,
                                 ap=[[NF, 8], [8 * NF, 16], [KT, NB], [1, KT]])
                    nc.sync.dma_start(out=d1, in_=q0i[:])
                    nc.sync.dma_start(out=dbg_q0.ap()[s],
                                      in_=q0i[:].rearrange("p b c -> p (b c)"))
                    idx_raw = midb.tile([128, 8, NB, KT], I16, tag="idx_raw")
                    d2src = bass.AP(tensor=q0_dram.ap().tensor,
                                    offset=q0_dram.ap().offset + s * 128 * NF,
                                    ap=[[0, 8], [8 * NF, 16], [1, 8 * NF]])
                    nc.sync.dma_start(
                        out=idx_raw[:].rearrange("p a b k -> p (a b k)"),
                        in_=d2src)
                    idx_w = persist.tile([128, KT, NCH, 64], I16, tag="idx_w",
                                         name=f"idxw{s}")
                    for ch in range(NCH):
                        dst = bass.AP(tensor=idx_w.tensor,
                                      offset=idx_w.offset + ch * 64,
                                      ap=[[idx_w.ap[0][0], 128], [NCH * 64, KT],
                                          [8, 8], [1, 8]])
                        srcap = bass.AP(tensor=idx_raw.tensor,
                                        offset=idx_raw.offset + (8 * ch) * KT,
                                        ap=[[idx_raw.ap[0][0], 128], [1, KT],
                                            [KT, 8], [NB * KT, 8]])
                        nc.vector.tensor_copy(out=dst, in_=srcap)
                    ctx_m2.__exit__(None, None, None)

                # ---- gather + combine + tap-contraction ----
                # gathers rotate over 4 SWDGE queues; an all-engine barrier per
                # group makes the cumulative DMA-sem waits order-safe, while
                # group g+1 gathers overlap group-g combines.
                psaT = ppa.tile([128, NB, CR], F32, tag="acc", name=f"psaT{s}")
                qn = 0
                G4s = {}

                def do_gathers(g):
                    nonlocal qn
                    for kk in range(GSZ):
                        k = GSZ * g + kk
                        if k >= KT:
                            continue
                        G4 = gpool.tile([128, NB, EL], BF16, tag="G4", bufs=4,
                                        name=f"G4_{s}_{k}")
                        for ch in range(NCH):
                            nc.gpsimd.dma_gather(
                                out_ap=G4[:, 8 * ch:8 * (ch + 1), :],
                                in_ap=table.ap()[s],
                                idxs_ap=idx_w[:, k, ch, :],
                                num_idxs=1024, num_idxs_reg=1024, elem_size=EL,
                                queue_num=qn % 4)
                            qn += 1
                        G4s[k] = G4

                do_gathers(0)
                for g in range(NG):
                    tc.strict_bb_all_engine_barrier()
                    if g + 1 < NG:
                        do_gathers(g + 1)
                    phi = gpool.tile([128, NB, GSZ, CPAD], BF16, tag="phi",
                                     name=f"phi{s}_{g}")
                    nc.vector.memset(phi[:], 0.0)
                    for kk in range(GSZ):
                        k = GSZ * g + kk
                        if k >= KT:
                            continue
                        G4 = G4s.pop(k)
                        pslc = phi[:, :, kk, 0:CR]
                        nc.scalar.copy(out=pslc, in_=G4[:, :, 0:CR])
                        for (bi, ut) in ((1, uyb), (2, uxb), (3, uxyb)):
                            ub = bass.AP(tensor=ut.tensor, offset=ut.offset + k,
                                         ap=[[ut.ap[0][0], 128], [KT, NB],
                                             [0, CR]])
                            tt = gpool.tile([128, NB, CR], BF16, tag="tt",
                                            name=f"tt{s}_{k}_{bi}")
                            nc.vector.tensor_tensor(
                                out=tt[:], in0=G4[:, :, CR * bi:CR * (bi + 1)],
                                in1=ub, op=OP.mult)
                            nc.vector.tensor_tensor(out=pslc, in0=pslc,
                                                    in1=tt[:], op=OP.add)
                    if g == 0:
                        nc.sync.dma_start(
                            out=dbg_phi.ap()[s],
                            in_=phi[:].rearrange("p b t c -> p (b t c)"))
                    for b in range(NB):
                        ptp = pps.tile([GSZ * CPAD, 128], BF16, tag="pst",
                                       name=f"ptp{s}_{g}_{b}")
                        nc.tensor.transpose(ptp[:], phi[:, b, :, :], idn_b[:])
                        phT = small.tile([GSZ * CPAD, 128], BF16, tag="phT",
                                         bufs=3, name=f"phT{s}_{g}_{b}")
                        nc.scalar.copy(out=phT[:], in_=ptp[:])
                        nc.tensor.matmul(psaT[:, b, :], phT[:], wdT[:, g, :],
                                         start=(g == 0), stop=(g == NG - 1))

                saT = persist2.tile([128, NB, CR], F32, tag="saT",
                                    name=f"saT{s}")
                nc.vector.tensor_copy(out=saT[:], in_=psaT[:])
                saT_all.append(saT)
                nc.sync.dma_start(out=dbg_sa.ap()[s],
                                  in_=saT[:].rearrange("p b c -> p (b c)"))

                # ---- SE gate ----
                cmax = small.tile([128, NB], F32, tag="cmax")
                nc.vector.tensor_reduce(out=cmax[:], in_=xrp[:],
                                        axis=mybir.AxisListType.X, op=OP.max)
                cm1 = small.tile([128, 1], F32, tag="cm1")
                nc.vector.tensor_reduce(out=cm1[:], in_=cmax[:],
                                        axis=mybir.AxisListType.X, op=OP.add)
                pchi = pps.tile([1, 1], F32, tag="ps", name=f"pchi{s}")
                nc.tensor.matmul(pchi[:], cm1[:], ones128[:], start=True,
                                 stop=True)
                chi = small.tile([1, 1], F32, tag="chi")
                nc.vector.tensor_copy(out=chi[:], in_=pchi[:])
                mlp_in = small.tile([CR, 2], BF16, tag="mlp_in")
                nc.scalar.activation(out=mlp_in[:, 0:1], in_=p_sum[:],
                                     func=AF.Copy, scale=1.0 / P)
                pbc = pps.tile([CR, 1], F32, tag="ps", name=f"pbc{s}")
                nc.tensor.matmul(pbc[:], ones_r[:, 0:CR], chi[:], start=True,
                                 stop=True)
                nc.scalar.activation(out=mlp_in[:, 1:2], in_=pbc[:], func=AF.Copy,
                                     scale=1.0 / P)
                ph1 = pps.tile([CR, 2], F32, tag="ps", name=f"ph1{s}")
                nc.tensor.matmul(ph1[:], wse1T[:], mlp_in[:], start=True,
                                 stop=True)
                h1 = small.tile([CR, 2], BF16, tag="h1")
                nc.scalar.activation(out=h1[:], in_=ph1[:], func=AF.Relu,
                                     bias=bse1[:])
                ph2 = pps.tile([CR, 2], F32, tag="ps", name=f"ph2{s}")
                nc.tensor.matmul(ph2[:], wse2T[:], h1[:], start=True, stop=True)
                se2 = small.tile([CR, 2], F32, tag="se2")
                nc.scalar.activation(out=se2[:], in_=ph2[:], func=AF.Sigmoid,
                                     bias=bse2[:])
                se_v = small.tile([CR, 1], F32, tag="se_v")
                nc.vector.tensor_tensor(out=se_v[:], in0=se2[:, 0:1],
                                        in1=se2[:, 1:2], op=OP.add)
                prow = pps.tile([1, CR], F32, tag="ps", name=f"prow{s}")
                nc.tensor.matmul(prow[:], se_v[:], idn_f[0:CR, 0:CR],
                                 start=True, stop=True)
                se_row = persist2.tile([1, CR], F32, tag="se_row",
                                       name=f"serow{s}")
                nc.vector.tensor_scalar(out=se_row[:], in0=prow[:],
                                        scalar1=float(wse_f), scalar2=None,
                                        op0=OP.mult)
                se_all.append(se_row)

                # ---- BN partial stats ----
                sq = small.tile([128, NB * CR], F32, tag="sq")
                nc.scalar.activation(
                    out=sq[:], in_=saT[:].rearrange("p b c -> p (b c)"),
                    func=AF.Square)
                s_sum = small.tile([1, NB * CR], F32, tag="s_sum")
                s_sq = small.tile([1, NB * CR], F32, tag="s_sq")
                for hh in range(2):
                    sl = slice(448 * hh, 448 * (hh + 1))
                    pst = pps.tile([1, 448], F32, tag="ps", name=f"pst{s}_{hh}")
                    nc.tensor.matmul(
                        pst[:], ones128[:],
                        saT[:].rearrange("p b c -> p (b c)")[:, sl],
                        start=True, stop=True)
                    nc.vector.tensor_copy(out=s_sum[:, sl], in_=pst[:])
                    pst2 = pps.tile([1, 448], F32, tag="ps", name=f"ps2{s}_{hh}")
                    nc.tensor.matmul(pst2[:], ones128[:], sq[:, sl],
                                     start=True, stop=True)
                    nc.vector.tensor_copy(out=s_sq[:, sl], in_=pst2[:])

                # fold over b -> [1, CR]
                def fold_b(t, nm):
                    cur, n = t, NB
                    while n > 1:
                        h = n // 2
                        nxt = small.tile([1, h, CR], F32, tag="foldb",
                                         name=f"fb_{nm}_{h}", bufs=2)
                        nc.vector.tensor_tensor(
                            out=nxt[:],
                            in0=bass.AP(tensor=cur.tensor, offset=cur.offset,
                                        ap=[[cur.ap[0][0], 1], [CR, h], [1, CR]]),
                            in1=bass.AP(tensor=cur.tensor,
                                        offset=cur.offset + h * CR,
                                        ap=[[cur.ap[0][0], 1], [CR, h], [1, CR]]),
                            op=OP.add)
                        cur, n = nxt, h
                    return cur
                f_sum = fold_b(s_sum, f"su{s}")
                f_sq = fold_b(s_sq, f"sq{s}")
                st_t = persist2.tile([1, 2, CR], F32, tag="st_t",
                                     name=f"stt{s}")
                nc.vector.tensor_copy(out=st_t[:, 0, :], in_=f_sum[:, 0, :])
                nc.vector.tensor_copy(out=st_t[:, 1, :], in_=f_sq[:, 0, :])
                stat_s.append(st_t)

            # ---- global BN stats (AllReduce) ----
            stat = small.tile([1, 64], F32, tag="stat")
            nc.vector.memset(stat[:], 0.0)
            nc.vector.tensor_tensor(out=stat[:, 0:CR], in0=stat_s[0][:, 0, :],
                                    in1=stat_s[1][:, 0, :], op=OP.add)
            nc.vector.tensor_tensor(out=stat[:, 32:32 + CR],
                                    in0=stat_s[0][:, 1, :],
                                    in1=stat_s[1][:, 1, :], op=OP.add)
            nc.sync.dma_start(out=ccin.ap(), in_=stat[:])
            nc.gpsimd.collective_compute(
                "AllReduce", OP.add, replica_groups=groups,
                ins=[ccin.ap()], outs=[ccout.ap()])
            statg = small.tile([1, 64], F32, tag="statg")
            nc.sync.dma_start(out=statg[:], in_=ccout.ap())

            Nt = float(B * P)
            mu = small.tile([1, CR], F32, tag="mu")
            nc.scalar.activation(out=mu[:], in_=statg[:, 0:CR], func=AF.Copy,
                                 scale=1.0 / Nt)
            e2 = small.tile([1, CR], F32, tag="e2")
            nc.scalar.activation(out=e2[:], in_=statg[:, 32:32 + CR],
                                 func=AF.Copy, scale=1.0 / Nt)
            mu2 = small.tile([1, CR], F32, tag="mu2")
            nc.vector.tensor_tensor(out=mu2[:], in0=mu[:], in1=mu[:], op=OP.mult)
            var = small.tile([1, CR], F32, tag="var")
            nc.vector.tensor_tensor(out=var[:], in0=e2[:], in1=mu2[:],
                                    op=OP.subtract)
            sd = small.tile([1, CR], F32, tag="sd")
            nc.vector.tensor_scalar(out=sd[:], in0=var[:], scalar1=1e-5,
                                    scalar2=None, op0=OP.add)
            nc.scalar.sqrt(out=sd[:], in_=sd[:])
            rsd = small.tile([1, CR], F32, tag="rsd")
            nc.vector.reciprocal(out=rsd[:], in_=sd[:])
            s_row = small.tile([1, CR], F32, tag="s_row")
            nc.vector.tensor_tensor(out=s_row[:], in0=gamma_r[:], in1=rsd[:],
                                    op=OP.mult)
            t_row = small.tile([1, CR], F32, tag="t_row")
            nc.vector.tensor_tensor(out=t_row[:], in0=mu[:], in1=s_row[:],
                                    op=OP.mult)
            nc.vector.tensor_tensor(out=t_row[:], in0=beta_r[:], in1=t_row[:],
                                    op=OP.subtract)

            def repl(row, tag, nm):
                pr = pps.tile([128, CR], F32, tag="ps", name=f"pr_{nm}")
                nc.tensor.matmul(pr[:], ones_r[:], row[:], start=True, stop=True)
                t = small.tile([128, CR], F32, tag=tag, name=f"rep_{nm}")
                nc.vector.tensor_copy(out=t[:], in_=pr[:])
                return t
            s_rep = repl(s_row, "s_rep", "s")
            t_rep = repl(t_row, "t_rep", "t")

            # ---- phase 2: BN apply, mix, restore ----
            for s in range(SLOC):
                ctx2 = contextlib.ExitStack()
                with ctx2:
                    ph2p = ctx2.enter_context(
                        tc.tile_pool(name=f"ph2_{s}", bufs=1))
                    saT = saT_all[s]
                    xrp = xrp_all[s]
                    se_rep = repl(se_all[s], "se_rep", f"se{s}")
                    a1 = ph2p.tile([128, NB, CR], F32, tag="a1")
                    srep_b = bass.AP(tensor=s_rep.tensor, offset=s_rep.offset,
                                     ap=[[s_rep.ap[0][0], 128], [0, NB], [1, CR]])
                    trep_b = bass.AP(tensor=t_rep.tensor, offset=t_rep.offset,
                                     ap=[[t_rep.ap[0][0], 128], [0, NB], [1, CR]])
                    nc.vector.tensor_tensor(out=a1[:], in0=saT[:], in1=srep_b,
                                            op=OP.mult)
                    nc.vector.tensor_tensor(out=a1[:], in0=a1[:], in1=trep_b,
                                            op=OP.add)
                    af = a1[:].rearrange("p b c -> p (b c)")
                    nc.scalar.activation(out=af, in_=af, func=AF.Relu)
                    nc.scalar.activation(out=af, in_=af, func=AF.Sigmoid)
                    u = ph2p.tile([128, NB, CR], F32, tag="u")
                    nc.vector.tensor_scalar(out=u[:], in0=a1[:],
                                            scalar1=float(wsa_f), scalar2=None,
                                            op0=OP.mult)
                    serep_b = bass.AP(tensor=se_rep.tensor,
                                      offset=se_rep.offset,
                                      ap=[[se_rep.ap[0][0], 128], [0, NB],
                                          [1, CR]])
                    nc.vector.tensor_tensor(out=u[:], in0=u[:], in1=serep_b,
                                            op=OP.add)
                    nc.vector.tensor_tensor(out=u[:], in0=u[:], in1=xrp[:],
                                            op=OP.mult)
                    mix_ch = ph2p.tile([CR, P], BF16, tag="mix_ch")
                    for b in range(NB):
                        pmt = pps.tile([CR, 128], F32, tag="ps",
                                       name=f"pmt{s}_{b}")
                        nc.tensor.transpose(pmt[:], u[:, b, :], idn_f[:])
                        nc.scalar.copy(out=mix_ch[:, 128 * b:128 * (b + 1)],
                                       in_=pmt[:])
                    for ccc in range(4):
                        for n in range(8):
                            pr = pps.tile([128, 512], F32, tag="ps",
                                          name=f"prr{s}_{ccc}_{n}")
                            nc.tensor.matmul(pr[:], wresT[:, ccc, :],
                                             mix_ch[:, 512 * n:512 * (n + 1)],
                                             start=True, stop=True)
                            ot = ph2p.tile([128, 512], F32, tag="ot", bufs=3,
                                           name=f"ot{s}_{ccc}_{n}")
                            nc.vector.tensor_copy(out=ot[:], in_=pr[:])
                            nc.sync.dma_start(
                                out=out_d.ap()[s, 128 * ccc:128 * (ccc + 1),
                                               512 * n:512 * (n + 1)],
                                in_=ot[:])
    nc.compile()
    return nc


def kernel(**inputs):
    x = np.asarray(inputs["x"], np.float32).reshape(B, C, P)
    consts = host_consts(
        np.asarray(inputs["w_red"], np.float32),
        np.asarray(inputs["w_se1"], np.float32),
        np.asarray(inputs["b_se1"], np.float32),
        np.asarray(inputs["w_se2"], np.float32),
        np.asarray(inputs["b_se2"], np.float32),
        np.asarray(inputs["w_off"], np.float32),
        np.asarray(inputs["b_off"], np.float32),
        np.asarray(inputs["w_dcn"], np.float32),
        np.asarray(inputs["bn_gamma"], np.float32),
        np.asarray(inputs["bn_beta"], np.float32),
        np.asarray(inputs["w_res"], np.float32),
        np.asarray(inputs["weight_se"], np.float32),
        np.asarray(inputs["weight_sa"], np.float32),
    )
    key = ("nc", consts["wse_f"], consts["wsa_f"])
    if key not in _cache:
        _cache[key] = build(local_bn=False, wse_f=consts["wse_f"],
                            wsa_f=consts["wsa_f"])
        _cache["nc"] = _cache[key]
    nc = _cache[key]
    base = {k: v for k, v in consts.items() if not isinstance(v, float)}
    in_maps = []
    for i in range(NCORES):
        m = dict(base)
        m["x"] = np.ascontiguousarray(x[SLOC * i:SLOC * (i + 1)])
        in_maps.append(m)
    res = run_bass_kernel_spmd(nc, in_maps, core_ids=list(range(NCORES)),
                               trace=False)
    out = np.concatenate([res.results[i]["out"] for i in range(NCORES)], axis=0)
    return out.reshape(B, C, H, W)

